# revision 1
# baseline (speedup 1.0000x reference)
"""Trainium2 Bass kernel for nn_LocalDenseCrossReadout.

Strategy:
- Data-parallel over batch: 8 batches -> 8 NeuronCores, one batch per core.
- Host-side (numpy, ~0.1% of FLOPs): FiLM conditioning (ctx -> gamma/beta),
  folding of LayerNorm affine + FiLM + score scale into the projection
  weights, and band-slicing of the additive mask.
- Device kernel per core: LayerNorm stats+apply for q [1024,512] and
  source [4096,512], transposed projections (f32r matmuls), banded local
  attention (768-wide aligned kv window per 128-row q tile), low-rank gate
  bias, softmax, attn@V and output projection.
"""

import sys

sys.path.insert(0, "/opt/trn_rl_repo")

import numpy as np

import concourse.bass as bass
import concourse.tile as tile
from concourse import bacc
from concourse import mybir
from concourse.bass_utils import run_bass_kernel_spmd
from concourse.masks import make_identity

DIM, QS, QT, KS, KT, WIN, B, RANK = 512, 64, 16, 256, 16, 4, 8, 32
Q = QS * QT  # 1024
K = KS * KT  # 4096
WINW = 768  # aligned kv window per 128-row q tile
NQT = Q // 128  # 8 q tiles
F32 = mybir.dt.float32
F32R = mybir.dt.float32r
FT = mybir.ActivationFunctionType
ALU = mybir.AluOpType
AX = mybir.AxisListType

# kv window start (aligned to 128) per q tile; phase split of the kv axis
WSTARTS = [0, 384, 896, 1408, 1920, 2432, 2944, 3328]
PHASES = [  # (kv_start, n_kv_tiles, q_tiles)
    (0, 17, range(0, 4)),
    (1920, 17, range(4, 8)),
]
KVW = 17 * 128  # 2176 kv columns held on-chip per phase


def r32(ap):
    return ap.bitcast(F32R)


def build_bass(debug=False, stage=5):
    nc = bacc.Bacc("TRN2", target_bir_lowering=False)
    q = nc.dram_tensor("q", [Q, DIM], F32, kind="ExternalInput")
    s = nc.dram_tensor("s", [K, DIM], F32, kind="ExternalInput")
    wq = nc.dram_tensor("wq", [DIM, DIM], F32R, kind="ExternalInput")
    wk = nc.dram_tensor("wk", [DIM, DIM], F32R, kind="ExternalInput")
    wv = nc.dram_tensor("wv", [DIM, DIM], F32R, kind="ExternalInput")
    wo = nc.dram_tensor("wo", [DIM, DIM], F32R, kind="ExternalInput")
    wgq = nc.dram_tensor("wgq", [DIM, RANK], F32R, kind="ExternalInput")
    wgk = nc.dram_tensor("wgk", [DIM, RANK], F32R, kind="ExternalInput")
    rqt = nc.dram_tensor("rqt", [128, 4], F32, kind="ExternalInput")
    rkt = nc.dram_tensor("rkt", [128, 4], F32, kind="ExternalInput")
    rv = nc.dram_tensor("rv", [1, DIM], F32R, kind="ExternalInput")
    bo = nc.dram_tensor("bo", [1, DIM], F32R, kind="ExternalInput")
    bmask = nc.dram_tensor("bmask", [NQT, 128, WINW], F32, kind="ExternalInput")
    out = nc.dram_tensor("out", [Q, DIM], F32, kind="ExternalOutput")
    if debug:
        d_qpT = nc.dram_tensor("d_qpT", [128, 4, Q], F32, kind="ExternalOutput")
        d_gq = nc.dram_tensor("d_gq", [32, Q], F32, kind="ExternalOutput")
        d_kT = nc.dram_tensor("d_kT", [128, 4, KVW], F32, kind="ExternalOutput")
        d_vb = nc.dram_tensor("d_vb", [128, 17, DIM], F32, kind="ExternalOutput")
        d_gk = nc.dram_tensor("d_gk", [32, KVW], F32, kind="ExternalOutput")
        d_S = nc.dram_tensor("d_S", [128, WINW], F32, kind="ExternalOutput")
        d_P = nc.dram_tensor("d_P", [128, WINW], F32, kind="ExternalOutput")
        d_oa = nc.dram_tensor("d_oa", [128, DIM], F32, kind="ExternalOutput")

    with tile.TileContext(nc) as tc:
        with (
            tc.tile_pool(name="consts", bufs=1) as consts,
            tc.tile_pool(name="wts", bufs=1) as wts,
            tc.tile_pool(name="kv", bufs=1) as kvpool,
            tc.tile_pool(name="xin", bufs=3) as xin,
            tc.tile_pool(name="stats", bufs=4) as stats,
            tc.tile_pool(name="xt", bufs=2) as xtp,
            tc.tile_pool(name="attn", bufs=2) as attn,
            tc.tile_pool(name="outp", bufs=2) as outp,
            tc.tile_pool(name="ps_s", bufs=3, space="PSUM") as ps_s,
            tc.tile_pool(name="ps_b", bufs=2, space="PSUM") as ps_b,
        ):
            # ---------------- constants ----------------
            ident = consts.tile([128, 128], F32)
            make_identity(nc, ident)
            eps = consts.tile([128, 1], F32)
            nc.vector.memset(eps, 1e-5)
            ones1 = consts.tile([1, 128], F32R)
            nc.vector.memset(ones1.bitcast(F32), 1.0)
            zero_c = consts.tile([128, 1], F32)
            nc.vector.memset(zero_c, 0.0)
            eps6 = consts.tile([128, 1], F32)
            nc.vector.memset(eps6, 1e-6)
            rqt_sb = consts.tile([128, 4], F32)
            nc.sync.dma_start(out=rqt_sb, in_=rqt[:, :])
            rkt_sb = consts.tile([128, 4], F32)
            nc.sync.dma_start(out=rkt_sb, in_=rkt[:, :])
            rv_sb = consts.tile([1, DIM], F32R)
            nc.sync.dma_start(out=rv_sb, in_=rv[:, :])
            bo_sb = consts.tile([1, DIM], F32R)
            nc.sync.dma_start(out=bo_sb, in_=bo[:, :])

            # weights as lhsT chunks: [128 (d_in in chunk c), c, d_out]
            def load_w(name, dram, n_out):
                t = wts.tile([128, 4, n_out], F32R, tag=name)
                for c in range(4):
                    nc.sync.dma_start(out=t[:, c, :], in_=dram[c * 128:(c + 1) * 128, :])
                return t

            wq_sb = load_w("wq", wq, DIM)
            wk_sb = load_w("wk", wk, DIM)
            wv_sb = load_w("wv", wv, DIM)
            wgq_sb = load_w("wgq", wgq, RANK)
            wgk_sb = load_w("wgk", wgk, RANK)

            # persistent activations
            qpT = kvpool.tile([128, 4, Q], F32R, tag="qpT")     # q_p^T chunks
            gq_sb = kvpool.tile([32, Q], F32R, tag="gq")        # gate_q^T

            # ---- LN + transpose of one 128-row tile into xt_big[:, :, j*128:] ----
            def ln_transpose(src_dram, row0, nrows, xt_big, jcol):
                x = xin.tile([128, DIM], F32, tag="x")
                nc.sync.dma_start(out=x[:nrows, :], in_=src_dram[row0:row0 + nrows, :])
                st6 = stats.tile([128, 6], F32, tag="st6")
                nc.vector.bn_stats(out=st6[:nrows], in_=x[:nrows, :])
                mv = stats.tile([128, 2], F32, tag="mv")
                nc.vector.bn_aggr(out=mv[:nrows], in_=st6[:nrows])
                sd = stats.tile([128, 1], F32, tag="sd")
                nc.scalar.activation(out=sd[:nrows], in_=mv[:nrows, 1:2],
                                     func=FT.Sqrt, bias=eps[:nrows], scale=1.0)
                rstd = stats.tile([128, 1], F32, tag="rstd")
                nc.vector.reciprocal(out=rstd[:nrows], in_=sd[:nrows])
                nmr = stats.tile([128, 1], F32, tag="nmr")
                nc.vector.scalar_tensor_tensor(
                    out=nmr[:nrows], in0=mv[:nrows, 0:1], scalar=-1.0,
                    in1=rstd[:nrows], op0=ALU.mult, op1=ALU.mult)
                xn = xin.tile([128, DIM], F32, tag="xn")
                nc.vector.tensor_scalar_mul(xn[:nrows], x[:nrows, :], rstd[:nrows])
                nc.vector.tensor_scalar_add(xn[:nrows], xn[:nrows], nmr[:nrows])
                tp = ps_s.tile([128, 4, 128], F32, tag="ps")
                for c in range(4):
                    nc.tensor.transpose(tp[:, c, :nrows], xn[:nrows, c * 128:(c + 1) * 128], ident)
                nc.vector.tensor_copy(xt_big[:, :, jcol * 128:jcol * 128 + nrows], tp[:, :, :nrows])

            # ---------------- phase A: queries ----------------
            for sup in range(2):  # 512 q rows each
                qt_big = xtp.tile([128, 4, 512], F32R, tag="xt_big")
                for j in range(4):
                    ln_transpose(q, sup * 512 + j * 128, 128, qt_big, j)
                # q_p^T chunks for these 512 q columns
                for m in range(4):
                    pp = ps_s.tile([128, 512], F32, tag="ps")
                    for c in range(4):
                        nc.tensor.matmul(pp, r32(wq_sb[:, c, m * 128:(m + 1) * 128]),
                                         r32(qt_big[:, c, :]), start=(c == 0), stop=(c == 3))
                    nc.scalar.activation(out=qpT[:, m, sup * 512:(sup + 1) * 512], in_=pp,
                                         func=FT.Identity, bias=rqt_sb[:, m:m + 1], scale=1.0)
                # gate_q^T = WgqS^T @ q_p^T (contraction over q_p feature dim)
                gp = ps_s.tile([32, 512], F32, tag="ps_g", bufs=1)
                for c in range(4):
                    nc.tensor.matmul(gp, r32(wgq_sb[:, c, :]),
                                     qpT[:, c, sup * 512:(sup + 1) * 512],
                                     start=(c == 0), stop=(c == 3))
                nc.vector.tensor_copy(gq_sb[:, sup * 512:(sup + 1) * 512], gp)

            # wo shares wq's slot; loaded after last wq use (phase A done)
            wo_sb = load_w("wq", wo, DIM)

            # ---------------- kv phases ----------------
            for kv_start, n_kv, q_tiles in PHASES:
                kT = kvpool.tile([128, 4, KVW], F32R, tag="kT")
                vb = kvpool.tile([128, 17, DIM], F32R, tag="vb")
                gk_sb = kvpool.tile([32, KVW], F32R, tag="gk")

                for sup in range(5):  # supertiles of 4,4,4,4,1 kv tiles
                    j0 = sup * 4
                    nt = min(4, n_kv - j0)
                    ncols = nt * 128
                    st_big = xtp.tile([128, 4, 512], F32R, tag="xt_big")
                    for j in range(nt):
                        ln_transpose(s, kv_start + (j0 + j) * 128, 128, st_big, j)
                    # k_p^T chunks
                    for m in range(4):
                        pp = ps_s.tile([128, 512], F32, tag="ps")
                        for c in range(4):
                            nc.tensor.matmul(pp[:, :ncols], r32(wk_sb[:, c, m * 128:(m + 1) * 128]),
                                             r32(st_big[:, c, :ncols]), start=(c == 0), stop=(c == 3))
                        nc.scalar.activation(out=kT[:, m, j0 * 128:j0 * 128 + ncols], in_=pp[:, :ncols],
                                             func=FT.Identity, bias=rkt_sb[:, m:m + 1], scale=1.0)
                    # v_p natural rows
                    for j in range(nt):
                        pv = ps_s.tile([128, 512], F32, tag="ps")
                        for c in range(4):
                            nc.tensor.matmul(pv, r32(st_big[:, c, j * 128:(j + 1) * 128]),
                                             r32(wv_sb[:, c, :]), start=(c == 0), stop=False)
                        nc.tensor.matmul(pv, r32(ones1), r32(rv_sb), start=False, stop=True)
                        nc.scalar.copy(vb[:, j0 + j, :], pv)
                    # gate_k^T = Wgk^T @ k_p^T
                    gp = ps_s.tile([32, 512], F32, tag="ps_g", bufs=1)
                    for c in range(4):
                        nc.tensor.matmul(gp[:, :ncols], r32(wgk_sb[:, c, :]),
                                         kT[:, c, j0 * 128:j0 * 128 + ncols],
                                         start=(c == 0), stop=(c == 3))
                    nc.vector.tensor_copy(gk_sb[:, j0 * 128:j0 * 128 + ncols], gp[:, :ncols])

                if debug and kv_start == 0:
                    nc.sync.dma_start(out=d_kT[:, :, :], in_=kT[:, :, :].bitcast(F32))
                    nc.sync.dma_start(out=d_vb[:, :, :], in_=vb[:, :, :].bitcast(F32))
                    nc.sync.dma_start(out=d_gk[:, :], in_=gk_sb[:, :].bitcast(F32))
                    nc.sync.dma_start(out=d_qpT[:, :, :], in_=qpT[:, :, :].bitcast(F32))
                    nc.sync.dma_start(out=d_gq[:, :], in_=gq_sb[:, :].bitcast(F32))

                # ---------------- attention over this phase's q tiles ----------------
                for t in q_tiles:
                    if stage < 3:
                        ob0 = outp.tile([128, DIM], F32, tag="ob")
                        nc.vector.tensor_copy(ob0, vb[:, 0, :].bitcast(F32))
                        nc.sync.dma_start(out=out[t * 128:(t + 1) * 128, :], in_=ob0)
                        continue
                    w0 = WSTARTS[t]
                    rel = w0 - kv_start
                    qc = bass.ts(t, 128)
                    msk = attn.tile([128, WINW], F32, tag="msk")
                    nc.sync.dma_start(out=msk, in_=bmask[t, :, :])
                    # gate logits -> gate bias
                    gl = ps_b.tile([128, WINW], F32, tag="ps_big")
                    for n0 in (0, 512):
                        nn_ = min(512, WINW - n0)
                        nc.tensor.matmul(gl[:, n0:n0 + nn_], r32(gq_sb[:, qc]),
                                         r32(gk_sb[:, rel + n0:rel + n0 + nn_]),
                                         start=True, stop=True)
                    if stage == 30:
                        obx = outp.tile([128, DIM], F32, tag="ob")
                        nc.vector.tensor_copy(obx, gl[:, :DIM])
                        nc.sync.dma_start(out=out[t * 128:(t + 1) * 128, :], in_=obx)
                        continue
                    sig = attn.tile([128, WINW], F32, tag="sig")
                    nc.scalar.activation(out=sig, in_=gl, func=FT.Sigmoid, bias=zero_c)
                    gb = attn.tile([128, WINW], F32, tag="gb")
                    nc.scalar.activation(out=gb, in_=sig, func=FT.Ln, bias=eps6, scale=1.0)
                    if stage == 31:
                        obx = outp.tile([128, DIM], F32, tag="ob")
                        nc.vector.tensor_copy(obx, gb[:, :DIM])
                        nc.sync.dma_start(out=out[t * 128:(t + 1) * 128, :], in_=obx)
                        continue
                    # scores
                    sc = ps_b.tile([128, WINW], F32, tag="ps_big")
                    for n0 in (0, 512):
                        nn_ = min(512, WINW - n0)
                        for c in range(4):
                            nc.tensor.matmul(sc[:, n0:n0 + nn_], r32(qpT[:, c, qc]),
                                             r32(kT[:, c, rel + n0:rel + n0 + nn_]),
                                             start=(c == 0), stop=(c == 3))
                    if stage == 32:
                        obx = outp.tile([128, DIM], F32, tag="ob")
                        nc.vector.tensor_copy(obx, sc[:, :DIM])
                        nc.sync.dma_start(out=out[t * 128:(t + 1) * 128, :], in_=obx)
                        continue
                    S = attn.tile([128, WINW], F32, tag="S")
                    nc.vector.scalar_tensor_tensor(out=S, in0=sc, scalar=1.0, in1=msk,
                                                   op0=ALU.mult, op1=ALU.add)
                    SG = attn.tile([128, WINW], F32, tag="sig")
                    nc.vector.tensor_add(SG, S, gb)
                    mx = stats.tile([128, 1], F32, tag="mx")
                    nc.vector.tensor_reduce(out=mx, in_=SG, axis=AX.X, op=ALU.max)
                    nmx = stats.tile([128, 1], F32, tag="nmx")
                    nc.vector.tensor_scalar_mul(nmx, mx, -1.0)
                    if debug and t == 0:
                        nc.sync.dma_start(out=d_S[:, :], in_=SG)
                    P = attn.tile([128, WINW], F32, tag="P")
                    nc.scalar.activation(out=P, in_=SG, func=FT.Exp, bias=nmx, scale=1.0)
                    rsum = stats.tile([128, 1], F32, tag="rsum")
                    nc.vector.tensor_reduce(out=rsum, in_=P, axis=AX.X, op=ALU.add)
                    rinv = stats.tile([128, 1], F32, tag="rinv")
                    nc.vector.reciprocal(out=rinv, in_=rsum)
                    if stage < 4 or stage == 33:
                        ob1 = outp.tile([128, DIM], F32, tag="ob")
                        nc.vector.tensor_copy(ob1, P[:, :DIM])
                        nc.sync.dma_start(out=out[t * 128:(t + 1) * 128, :], in_=ob1)
                        continue
                    # attn^T (unnormalized)
                    pt = ps_b.tile([128, WINW], F32, tag="ps_big")
                    for cc in range(6):
                        nc.tensor.transpose(pt[:, cc * 128:(cc + 1) * 128],
                                            P[:, cc * 128:(cc + 1) * 128], ident)
                    aT = attn.tile([128, 6, 128], F32R, tag="aT")
                    nc.vector.tensor_copy(aT, pt.rearrange("p (a b) -> p a b", a=6))
                    # attn @ V
                    av = ps_s.tile([128, 512], F32, tag="ps")
                    for cc in range(6):
                        nc.tensor.matmul(av, r32(aT[:, cc, :]), r32(vb[:, rel // 128 + cc, :]),
                                         start=(cc == 0), stop=(cc == 5))
                    oa = outp.tile([128, DIM], F32, tag="oa")
                    nc.vector.tensor_scalar_mul(oa, av, rinv)  # normalize rows
                    if debug and t == 0:
                        nc.sync.dma_start(out=d_P[:, :], in_=P)
                        nc.sync.dma_start(out=d_oa[:, :], in_=oa)
                    if stage < 5:
                        nc.sync.dma_start(out=out[t * 128:(t + 1) * 128, :], in_=oa)
                        continue
                    # out = oa @ Wo + bo
                    ot = ps_s.tile([128, 4, 128], F32, tag="ps")
                    for c in range(4):
                        nc.tensor.transpose(ot[:, c, :], oa[:, c * 128:(c + 1) * 128], ident)
                    oaT = outp.tile([128, 4, 128], F32R, tag="oaT")
                    nc.vector.tensor_copy(oaT, ot)
                    fin = ps_s.tile([128, 512], F32, tag="ps")
                    for c in range(4):
                        nc.tensor.matmul(fin, r32(oaT[:, c, :]), r32(wo_sb[:, c, :]),
                                         start=(c == 0), stop=False)
                    nc.tensor.matmul(fin, r32(ones1), r32(bo_sb), start=False, stop=True)
                    ob = outp.tile([128, DIM], F32, tag="ob")
                    nc.vector.tensor_copy(ob, fin)
                    nc.sync.dma_start(out=out[t * 128:(t + 1) * 128, :], in_=ob)

    if not nc.is_finalized():
        nc.finalize()
    return nc


_NC_CACHE = None


def _get_nc():
    global _NC_CACHE
    if _NC_CACHE is None:
        _NC_CACHE = build_bass()
    return _NC_CACHE


def _host_fold(inputs):
    f32 = np.float32
    scale = f32(DIM ** -0.5)
    ctx0 = np.asarray(inputs["ctx0"], f32)
    ctx1 = np.asarray(inputs["ctx1"], f32)
    pre = ctx0 @ inputs["Wc0"] + inputs["bc0"] + ctx1 @ inputs["Wc1"] + inputs["bc1"]
    pre = np.asarray(pre, f32)
    h = pre / (1.0 + np.exp(-pre))
    gb = np.asarray(h @ inputs["Wf"] + inputs["bf"], f32)
    gamma, beta = gb[:, :DIM], gb[:, DIM:]

    qn_g = np.asarray(inputs["qn_g"], f32)
    qn_b = np.asarray(inputs["qn_b"], f32)
    kvn_g = np.asarray(inputs["kvn_g"], f32)
    kvn_b = np.asarray(inputs["kvn_b"], f32)
    Wq, bq = np.asarray(inputs["Wq"], f32), np.asarray(inputs["bq"], f32)
    Wk, bk = np.asarray(inputs["Wk"], f32), np.asarray(inputs["bk"], f32)
    Wv, bv = np.asarray(inputs["Wv"], f32), np.asarray(inputs["bv"], f32)
    mask = np.asarray(inputs["mask"], f32)

    WkS = np.ascontiguousarray((Wk * kvn_g[:, None]).astype(f32))
    r_k = (kvn_b @ Wk + bk).astype(f32)
    WvS = np.ascontiguousarray((Wv * kvn_g[:, None]).astype(f32))
    r_v = (kvn_b @ Wv + bv).astype(f32)
    WgqS = np.ascontiguousarray((inputs["Wgq"] / scale / np.sqrt(RANK)).astype(f32))
    Wgk = np.ascontiguousarray(np.asarray(inputs["Wgk"], f32))
    Wo = np.ascontiguousarray(np.asarray(inputs["Wo"], f32))
    bo = np.asarray(inputs["bo"], f32)

    bmask = np.stack([mask[t * 128:(t + 1) * 128, w:w + WINW]
                      for t, w in enumerate(WSTARTS)]).astype(f32)
    bmask = np.ascontiguousarray(np.maximum(bmask, -1e30))  # avoid -inf on device

    query = np.asarray(inputs["query"], f32).reshape(B, Q, DIM)
    source = np.asarray(inputs["source"], f32).reshape(B, K, DIM)

    in_maps = []
    for b in range(B):
        sg = (qn_g * (1.0 + gamma[b])).astype(f32)
        WqS = np.ascontiguousarray((Wq * sg[:, None] * scale).astype(f32))
        r_q = (((qn_b * (1.0 + gamma[b]) + beta[b]) @ Wq + bq) * scale).astype(f32)
        in_maps.append({
            "q": np.ascontiguousarray(query[b]),
            "s": np.ascontiguousarray(source[b]),
            "wq": WqS, "wk": WkS, "wv": WvS, "wo": Wo,
            "wgq": WgqS, "wgk": Wgk,
            "rqt": np.ascontiguousarray(r_q.reshape(4, 128).T),
            "rkt": np.ascontiguousarray(r_k.reshape(4, 128).T),
            "rv": r_v.reshape(1, DIM),
            "bo": bo.reshape(1, DIM),
            "bmask": bmask,
        })
    return in_maps


def kernel(**inputs):
    nc = _get_nc()
    in_maps = _host_fold(inputs)
    res = run_bass_kernel_spmd(nc, in_maps, core_ids=list(range(B)))
    out = np.stack([res.results[b]["out"] for b in range(B)])
    return out.reshape(B, QS, QT, DIM).astype(np.float32)


if __name__ == "__main__":
    build_bass()
    print("bass build OK")



# revision 3
# speedup vs baseline: 1.1490x; 1.1490x over previous
"""Trainium2 Bass kernel for nn_LocalDenseCrossReadout (optimized v3).

Key design (vs v1 baseline 252-292us, v2 334us):
- Data-parallel over batch: 8 batches -> 8 NeuronCores.
- All matmul operands bf16 (f32 PSUM accumulation). End-to-end rel err
  ~4e-3 vs the 2e-2 gate.
- Host folds: LN affine + FiLM + score scale into Wq; Wvo = Wv@Wo removes
  the output projection (out = attn @ (s_base@Wvo) + obias since
  sum(attn)=1 passes v-bias and bo through softmax).
- Single activation-table set for the whole kernel (ACT_TABLE_LOAD thrash
  in v1/v2 cost 25-41us): scalar engine runs only Exp/Tanh/Identity/Copy
  (one set). rstd = magic-rsqrt + 2 Newton iterations on DVE (int ALU).
  gate sigmoid = 0.5*tanh(x/2)+0.5.
- Multiplicative {0,1} mask applied to the gate (P = (sig+1e-6)*m01 * E):
  exp reads scores straight from PSUM, no additive-mask pass.
- Rowsum fused into the P scalar_tensor_tensor via accum_out; softmax
  needs no max-subtraction (|scores| <= ~1.6, validated).
- PE transposes (bf16, 1 cyc/row) instead of v2's slow DMA xbar (1.2us).
- Software-pipelined emission: prep(g+1) [load/stats/rstd/apply/transpose]
  is emitted before proj(g) [projection matmuls + psum copies] so each
  in-order engine sees early-stage work before late-stage work (no
  convoys). Attention tiles interleave with kv groups.
- Engine assignment: DVE=bn_stats/newton/xt+PT copies/P+ob fusions;
  Scalar=LN applies(Identity w/ scale+bias APs)/kT+vb+gate copies/
  Tanh/Exp; GpSimd=gate masking. DMA queues: sync=x loads+stores,
  scalar=weights+masks.
"""

import sys

sys.path.insert(0, "/opt/trn_rl_repo")

import numpy as np
import ml_dtypes

import concourse.bass as bass
import concourse.tile as tile
from concourse import bacc
from concourse import mybir
from concourse.bass_utils import run_bass_kernel_spmd
from concourse.masks import make_identity

DIM, QS, QT, KS, KT, WIN, B, RANK = 512, 64, 16, 256, 16, 4, 8, 32
Q = QS * QT   # 1024
K = KS * KT   # 4096
WINW = 768    # aligned kv window per 128-row q tile
NQT = Q // 128    # 8 q tiles
NKVT = K // 128   # 32 kv tiles
F32 = mybir.dt.float32
I32 = mybir.dt.int32
BF16 = mybir.dt.bfloat16
FT = mybir.ActivationFunctionType
ALU = mybir.AluOpType
bfloat16 = ml_dtypes.bfloat16

# kv window start (aligned to 128) per q tile
WSTARTS = [0, 384, 896, 1408, 1920, 2432, 2944, 3328]


def build_bass():
    nc = bacc.Bacc("TRN2", target_bir_lowering=False)
    q = nc.dram_tensor("q", [Q, DIM], F32, kind="ExternalInput")
    s = nc.dram_tensor("s", [K, DIM], F32, kind="ExternalInput")
    wq = nc.dram_tensor("wq", [DIM, DIM], BF16, kind="ExternalInput")
    wk = nc.dram_tensor("wk", [DIM, DIM], BF16, kind="ExternalInput")
    wvo = nc.dram_tensor("wvo", [DIM, DIM], BF16, kind="ExternalInput")
    wgq = nc.dram_tensor("wgq", [DIM, RANK], BF16, kind="ExternalInput")
    wgk = nc.dram_tensor("wgk", [DIM, RANK], BF16, kind="ExternalInput")
    rqt = nc.dram_tensor("rqt", [128, 4], F32, kind="ExternalInput")
    rkt = nc.dram_tensor("rkt", [128, 4], F32, kind="ExternalInput")
    obias = nc.dram_tensor("obias", [128, DIM], F32, kind="ExternalInput")
    m01 = nc.dram_tensor("m01", [NQT, 128, WINW], BF16, kind="ExternalInput")
    out = nc.dram_tensor("out", [Q, DIM], F32, kind="ExternalOutput")

    with tile.TileContext(nc) as tc:
        with (
            tc.tile_pool(name="consts", bufs=1) as consts,
            tc.tile_pool(name="wts", bufs=1) as wts,
            tc.tile_pool(name="big", bufs=1) as big,
            tc.tile_pool(name="xin", bufs=8) as xin,
            tc.tile_pool(name="stats", bufs=2) as stats,
            tc.tile_pool(name="nmrp", bufs=8) as nmrp,
            tc.tile_pool(name="xnp", bufs=6) as xnp,
            tc.tile_pool(name="xtp", bufs=3) as xtp,
            tc.tile_pool(name="attn", bufs=2) as attn,
            tc.tile_pool(name="outp", bufs=2) as outp,
            tc.tile_pool(name="ps_mm", bufs=2, space="PSUM") as ps_mm,
            tc.tile_pool(name="ps_big", bufs=2, space="PSUM") as ps_big,
            tc.tile_pool(name="ps_tx", bufs=1, space="PSUM") as ps_tx,
            tc.tile_pool(name="ps_tp", bufs=1, space="PSUM") as ps_tp,
        ):
            # ---------------- constants / weights ----------------
            rqt_sb = consts.tile([128, 4], F32)
            nc.sync.dma_start(out=rqt_sb, in_=rqt[:, :])
            rkt_sb = consts.tile([128, 4], F32)
            nc.sync.dma_start(out=rkt_sb, in_=rkt[:, :])
            obias_sb = consts.tile([128, DIM], F32)
            nc.sync.dma_start(out=obias_sb, in_=obias[:, :])
            ident = consts.tile([128, 128], BF16)
            make_identity(nc, ident)

            def load_w(name, dram, n_out):
                t = wts.tile([128, 4, n_out], BF16, tag=name, name=name)
                for c in range(4):
                    nc.scalar.dma_start(out=t[:, c, :],
                                        in_=dram[c * 128:(c + 1) * 128, :])
                return t

            # persistent activations (all bf16)
            qpT = big.tile([128, 4, Q], BF16, name="qpT")
            gq_sb = big.tile([32, Q], BF16, name="gq_sb")
            kT = big.tile([128, 4, K], BF16, name="kT")
            vb = big.tile([128, NKVT, DIM], BF16, name="vb")
            gk_sb = big.tile([32, K], BF16, name="gk_sb")

            # rstd = (var+eps)^-0.5 entirely on DVE: magic-constant rsqrt
            # + 2 Newton steps (keeps ScalarE on one activation-table set)
            def rsqrt4(var4):
                v = stats.tile([128, 4], F32, tag="nv", name="nv")
                nc.vector.tensor_scalar(out=v, in0=var4, scalar1=1e-5,
                                        scalar2=None, op0=ALU.add)
                half = stats.tile([128, 4], F32, tag="half", name="half")
                nc.vector.tensor_scalar(out=half.bitcast(I32), in0=v.bitcast(I32),
                                        scalar1=1, scalar2=None,
                                        op0=ALU.arith_shift_right)
                r = stats.tile([128, 4], F32, tag="r0", name="r0")
                nc.vector.tensor_scalar(out=r.bitcast(I32), in0=half.bitcast(I32),
                                        scalar1=0x5f3759df, scalar2=-1,
                                        op0=ALU.subtract, op1=ALU.mult)
                for it in range(2):
                    t1 = stats.tile([128, 4], F32, tag=f"t1_{it}", name=f"t1_{it}")
                    nc.vector.tensor_tensor(out=t1, in0=v, in1=r, op=ALU.mult)
                    t2 = stats.tile([128, 4], F32, tag=f"t2_{it}", name=f"t2_{it}")
                    nc.vector.tensor_tensor(out=t2, in0=t1, in1=r, op=ALU.mult)
                    t3 = stats.tile([128, 4], F32, tag=f"t3_{it}", name=f"t3_{it}")
                    nc.vector.tensor_scalar(out=t3, in0=t2, scalar1=-0.5,
                                            scalar2=1.5, op0=ALU.mult, op1=ALU.add)
                    rn = stats.tile([128, 4], F32, tag=f"rn_{it}", name=f"rn_{it}")
                    nc.vector.tensor_tensor(out=rn, in0=r, in1=t3, op=ALU.mult)
                    r = rn
                return r

            # ---- prep stage: load, stats, rstd, LN-apply, transpose ----
            def prep(src_dram, g):
                mv4 = stats.tile([128, 4, 2], F32, tag="mv4", name="mv4")
                xs = []
                for j in range(4):
                    x = xin.tile([128, DIM], F32, tag="x", name="x")
                    nc.sync.dma_start(out=x, in_=src_dram[(g * 4 + j) * 128:
                                                          (g * 4 + j + 1) * 128, :])
                    xs.append(x)
                    st6 = stats.tile([128, 6], F32, tag=f"st6_{j}", name=f"st6_{j}")
                    nc.vector.bn_stats(out=st6, in_=x)
                    nc.vector.bn_aggr(out=mv4[:, j, :], in_=st6)
                rstd4 = rsqrt4(mv4[:, :, 1])
                xt_group = xtp.tile([128, 4, 4, 128], BF16, tag="xtg", name="xtg")
                for j in range(4):
                    nmr = nmrp.tile([128, 1], F32, tag="nmr", name="nmr")
                    nc.vector.scalar_tensor_tensor(
                        out=nmr, in0=mv4[:, j, 0:1], scalar=-1.0,
                        in1=rstd4[:, j:j + 1], op0=ALU.mult, op1=ALU.mult)
                    xn = xnp.tile([128, DIM], BF16, tag="xn", name="xn")
                    nc.scalar.activation(out=xn, in_=xs[j], func=FT.Identity,
                                         bias=nmr, scale=rstd4[:, j:j + 1])
                    xtT = ps_tx.tile([128, 4, 128], BF16, tag="xtT", name="xtT")
                    for c in range(4):
                        nc.tensor.transpose(xtT[:, c, :],
                                            xn[:, c * 128:(c + 1) * 128], ident)
                    nc.vector.tensor_copy(xt_group[:, j, :, :], xtT)
                return xt_group

            # ---- proj stage: projections + psum->sbuf copies ----
            def proj(xt_group, g, w_sb, r_sb, pT, gw_sb, g_dst, vdst):
                c0 = g * 512
                for m in range(4):
                    pp = ps_mm.tile([128, 512], F32, tag="mm", name="pp")
                    for c in range(4):
                        nc.tensor.matmul(pp, w_sb[:, c, m * 128:(m + 1) * 128],
                                         xt_group[:, :, c, :],
                                         start=(c == 0), stop=(c == 3))
                    nc.scalar.activation(out=pT[:, m, c0:c0 + 512], in_=pp,
                                         func=FT.Identity, bias=r_sb[:, m:m + 1])
                gpt = ps_mm.tile([128, 512], F32, tag="mm", name="gpt")
                gp = gpt[0:32, :]
                for c in range(4):
                    nc.tensor.matmul(gp, gw_sb[:, c, :], pT[:, c, c0:c0 + 512],
                                     start=(c == 0), stop=(c == 3))
                nc.scalar.copy(g_dst[:, c0:c0 + 512], gp)
                if vdst is not None:
                    for j in range(4):
                        pv = ps_mm.tile([128, 512], F32, tag="mm", name="pv")
                        for c in range(4):
                            nc.tensor.matmul(pv, xt_group[:, j, c, :],
                                             wvo_sb[:, c, :],
                                             start=(c == 0), stop=(c == 3))
                        nc.vector.tensor_copy(vdst[:, g * 4 + j, :], pv)

            # ---- attention for q tile t ----
            def do_attn(t):
                w0 = WSTARTS[t]
                jb = w0 // 128
                qc = bass.ts(t, 128)
                msk = attn.tile([128, WINW], BF16, tag="msk", name="msk")
                nc.sync.dma_start(out=msk, in_=m01[t, :, :])
                # gate logits -> sigmoid via tanh (same act table set as exp)
                gl = ps_big.tile([128, WINW], F32, tag="big", name="gl")
                for n0 in (0, 512):
                    nn_ = min(512, WINW - n0)
                    nc.tensor.matmul(gl[:, n0:n0 + nn_], gq_sb[:, qc],
                                     gk_sb[:, w0 + n0:w0 + n0 + nn_],
                                     start=True, stop=True)
                T = attn.tile([128, WINW], F32, tag="T", name="T")
                nc.scalar.activation(out=T, in_=gl, func=FT.Tanh, scale=0.5)
                G = attn.tile([128, WINW], F32, tag="G", name="G")
                nc.vector.tensor_scalar(out=G, in0=T, scalar1=0.5,
                                        scalar2=0.500001, op0=ALU.mult,
                                        op1=ALU.add)
                GM = attn.tile([128, WINW], F32, tag="GM", name="GM")
                nc.gpsimd.tensor_tensor(out=GM, in0=G, in1=msk, op=ALU.mult)
                # scores
                sc = ps_big.tile([128, WINW], F32, tag="big", name="sc")
                for n0 in (0, 512):
                    nn_ = min(512, WINW - n0)
                    for c in range(4):
                        nc.tensor.matmul(sc[:, n0:n0 + nn_], qpT[:, c, qc],
                                         kT[:, c, w0 + n0:w0 + n0 + nn_],
                                         start=(c == 0), stop=(c == 3))
                E = attn.tile([128, WINW], F32, tag="E", name="E")
                nc.scalar.activation(out=E, in_=sc, func=FT.Exp)
                # P = GM * E (bf16) with fused rowsum
                P = attn.tile([128, WINW], BF16, tag="P", name="P")
                rsum = nmrp.tile([128, 1], F32, tag="rsum", name="rsum")
                nc.vector.scalar_tensor_tensor(out=P, in0=GM, scalar=1.0, in1=E,
                                               op0=ALU.mult, op1=ALU.mult,
                                               accum_out=rsum)
                rinv = nmrp.tile([128, 1], F32, tag="rinv", name="rinv")
                nc.vector.reciprocal(out=rinv, in_=rsum)
                # P^T (PE, bf16)
                ptT = ps_tp.tile([128, 8, 128], BF16, tag="ptT", name="ptT")
                for cc in range(6):
                    nc.tensor.transpose(ptT[:, cc, :],
                                        P[:, cc * 128:(cc + 1) * 128], ident)
                PT = attn.tile([128, 6, 128], BF16, tag="PT", name="PT")
                nc.vector.tensor_copy(PT, ptT[:, 0:6, :])
                # attn @ V
                av = ps_mm.tile([128, 512], F32, tag="mm", name="av")
                for cc in range(6):
                    nc.tensor.matmul(av, PT[:, cc, :], vb[:, jb + cc, :],
                                     start=(cc == 0), stop=(cc == 5))
                ob = outp.tile([128, DIM], F32, tag="ob", name="ob")
                nc.vector.scalar_tensor_tensor(out=ob, in0=av, scalar=rinv,
                                               in1=obias_sb, op0=ALU.mult,
                                               op1=ALU.add)
                nc.scalar.dma_start(out=out[t * 128:(t + 1) * 128, :], in_=ob)

            # ---------------- software-pipelined schedule ----------------
            wq_sb = load_w("wq", wq, DIM)
            wgq_sb = load_w("wgq", wgq, RANK)
            wk_sb = load_w("wk", wk, DIM)
            wvo_sb = load_w("wvo", wvo, DIM)
            wgk_sb = load_w("wgk", wgk, RANK)

            # stage list: (kind, arg); proj runs two stages behind prep
            stages = []
            for g in range(2):
                stages.append(("q", g))
            for g in range(8):
                stages.append(("s", g))
            attn_after = {}
            for t in range(NQT):
                gneed = (WSTARTS[t] + WINW - 1) // 512
                attn_after.setdefault(gneed, []).append(t)

            def run_proj(kind, g, xt):
                if kind == "q":
                    proj(xt, g, wq_sb, rqt_sb, qpT, wgq_sb, gq_sb, None)
                else:
                    proj(xt, g, wk_sb, rkt_sb, kT, wgk_sb, gk_sb, vb)
                    for t in attn_after.get(g, []):
                        do_attn(t)

            window = []  # [(kind, g, xt_group)] awaiting proj
            STAGGER = 2
            for kind, g in stages:
                src = q if kind == "q" else s
                window.append((kind, g, prep(src, g)))
                if len(window) > STAGGER:
                    pk, pg, pxt = window.pop(0)
                    run_proj(pk, pg, pxt)
            for pk, pg, pxt in window:
                run_proj(pk, pg, pxt)

    if not nc.is_finalized():
        nc.finalize()
    return nc


_NC_CACHE = None


def _get_nc():
    global _NC_CACHE
    if _NC_CACHE is None:
        _NC_CACHE = build_bass()
    return _NC_CACHE


def _host_fold(inputs):
    f32 = np.float32
    scale = f32(DIM ** -0.5)
    ctx0 = np.asarray(inputs["ctx0"], f32)
    ctx1 = np.asarray(inputs["ctx1"], f32)
    pre = ctx0 @ inputs["Wc0"] + inputs["bc0"] + ctx1 @ inputs["Wc1"] + inputs["bc1"]
    pre = np.asarray(pre, f32)
    h = pre / (1.0 + np.exp(-pre))
    gb = np.asarray(h @ inputs["Wf"] + inputs["bf"], f32)
    gamma, beta = gb[:, :DIM], gb[:, DIM:]

    qn_g = np.asarray(inputs["qn_g"], f32)
    qn_b = np.asarray(inputs["qn_b"], f32)
    kvn_g = np.asarray(inputs["kvn_g"], f32)
    kvn_b = np.asarray(inputs["kvn_b"], f32)
    Wq, bq = np.asarray(inputs["Wq"], f32), np.asarray(inputs["bq"], f32)
    Wk, bk = np.asarray(inputs["Wk"], f32), np.asarray(inputs["bk"], f32)
    Wv, bv = np.asarray(inputs["Wv"], f32), np.asarray(inputs["bv"], f32)
    Wo, bo = np.asarray(inputs["Wo"], f32), np.asarray(inputs["bo"], f32)
    mask = np.asarray(inputs["mask"], f32)

    WkS = (Wk * kvn_g[:, None]).astype(f32)
    r_k = (kvn_b @ Wk + bk).astype(f32)
    Wvo = ((Wv * kvn_g[:, None]) @ Wo).astype(f32)
    r_vo = ((kvn_b @ Wv + bv) @ Wo + bo).astype(f32)
    WgqS = (inputs["Wgq"] / scale / np.sqrt(RANK)).astype(f32)
    Wgk = np.asarray(inputs["Wgk"], f32)

    def b16(a):
        return np.ascontiguousarray(np.asarray(a, f32).astype(bfloat16))

    m01 = np.stack([(mask[t * 128:(t + 1) * 128, w:w + WINW] == 0.0)
                    for t, w in enumerate(WSTARTS)]).astype(f32)
    m01 = b16(m01)

    obias = np.ascontiguousarray(np.tile(r_vo.reshape(1, DIM), (128, 1)).astype(f32))

    query = np.asarray(inputs["query"], f32).reshape(B, Q, DIM)
    source = np.asarray(inputs["source"], f32).reshape(B, K, DIM)

    wk_b = b16(WkS)
    wvo_b = b16(Wvo)
    wgq_b = b16(WgqS)
    wgk_b = b16(Wgk)
    rkt = np.ascontiguousarray(r_k.reshape(4, 128).T.astype(f32))

    in_maps = []
    for b in range(B):
        sg = (qn_g * (1.0 + gamma[b])).astype(f32)
        WqS = (Wq * sg[:, None] * scale).astype(f32)
        r_q = (((qn_b * (1.0 + gamma[b]) + beta[b]) @ Wq + bq) * scale).astype(f32)
        in_maps.append({
            "q": np.ascontiguousarray(query[b]),
            "s": np.ascontiguousarray(source[b]),
            "wq": b16(WqS), "wk": wk_b, "wvo": wvo_b,
            "wgq": wgq_b, "wgk": wgk_b,
            "rqt": np.ascontiguousarray(r_q.reshape(4, 128).T.astype(f32)),
            "rkt": rkt,
            "obias": obias,
            "m01": m01,
        })
    return in_maps


def kernel(**inputs):
    nc = _get_nc()
    in_maps = _host_fold(inputs)
    res = run_bass_kernel_spmd(nc, in_maps, core_ids=list(range(B)))
    out = np.stack([res.results[b]["out"] for b in range(B)])
    return out.reshape(B, QS, QT, DIM).astype(np.float32)


if __name__ == "__main__":
    build_bass()
    print("bass build OK")
